# revision 28
# baseline (speedup 1.0000x reference)
"""MoE soft-routing MLP kernel for 8 Trainium2 NeuronCores.

Reference computation (per layer l, weights a_l: [E, out, in], bias b_l: [E, out]):
    y_e = H @ a_e^T + b_e          # per-expert GEMM      [B, out]
    H'  = sum_e wb[e, :, None] * y_e                      [B, out]
    H'  = elu(H') for layers 0, 1

Distribution: data-parallel over batch B=4096 across 8 cores (B_loc=512).
Expert weights are replicated to every core; x and weight_blend are sharded
along batch.

Per-core algorithm (all activations kept TRANSPOSED on chip: [feature, batch]):
    out[o, b] = sum_e sum_i aT_e[i, o] * (wb[e, b] * Ht[i, b])  + sum_e b_e[o] wb[e, b]
  - the bias term is a K=8 matmul (lhsT = beta [E, o-chunk], rhs = wb [E, b])
    that seeds each PSUM bank (start=True),
  - each expert's contribution accumulates into the same PSUM bank:
    lhsT = aT_e[i-tile, o-chunk] (128x128 stationary),
    rhs  = Zt_e[i-tile] = Ht[i-tile] * bcast(wb[e, :]) (128x512 moving),
  - blend weights arrive pre-broadcast from the host ([E, 128, B_LOC]).
  - ELU+1 is evicted as relu(x) + min(exp(x), 1)  (= elu(x) + 1; valid since
    the preactivations here are far below exp-overflow), and the -1 folds
    into the next layer's blend: zt = (h1 - 1) * wbb_e, one DVE op.

Matmuls run in float32r (TF32-like, 1 cycle/row at N=512 vs 4 for fp32;
measured rel-err ~1.6e-4 per K=128 matmul). Everything else is fp32.

The output of the final layer is DMA'd out still transposed ([512, 512] per
core) and un-transposed on the host.
"""

import os
import sys

if "/opt/trn_rl_repo" not in sys.path:
    sys.path.insert(0, "/opt/trn_rl_repo")

import numpy as np

import concourse.bass as bass  # noqa: F401  (bass must import before mybir use)
import concourse.mybir as mybir
import concourse.tile as tile
from concourse import bacc
from concourse.bass_utils import run_bass_kernel_spmd

F32 = mybir.dt.float32
F32R = mybir.dt.float32r
F16 = mybir.dt.float16
AF = mybir.ActivationFunctionType
ALU = mybir.AluOpType

# Matmul operand dtype: "f32r" (TF32-like, fp32 bytes in DRAM) or "f16"
# (half the weight DMA; weights pre-scaled by 2^WEXP and blend weights by
# 2^ZEXP on the host to stay out of fp16-subnormal range, descaled by
# 2^-(WEXP+ZEXP) in the PSUM-eviction activations).
MM_MODE = os.environ.get("BASS_MM_MODE", "f16")
MM_DT = F16 if MM_MODE == "f16" else F32R
WEXP, ZEXP = (8, 6) if MM_MODE == "f16" else (0, 0)
DESCALE = float(2.0 ** -(WEXP + ZEXP))

B, E = 4096, 8
DIMS = [512, 1024, 1024, 512]
N_CORES = 8
B_LOC = B // N_CORES  # 512; also the matmul moving free-dim (max for 4-byte)
P = 128

# (in, out, apply_elu) per layer
LAYERS = [
    (DIMS[0], DIMS[1], True),
    (DIMS[1], DIMS[2], True),
    (DIMS[2], DIMS[3], False),
]

LAST_RESULTS = None  # BassKernelResults of the most recent run (for test.py)
_NC_CACHE = None


def _build():
    nc = bacc.Bacc(None, target_bir_lowering=False, debug=False)

    xt = nc.dram_tensor("xt", [DIMS[0], B_LOC], F32, kind="ExternalInput")
    wb = nc.dram_tensor("wb", [E, B_LOC], MM_DT, kind="ExternalInput")
    wbbd = nc.dram_tensor("wbb", [E, P, B_LOC], F32, kind="ExternalInput")
    ats = [
        nc.dram_tensor(f"a{l}t", [E, din, dout], MM_DT, kind="ExternalInput")
        for l, (din, dout, _) in enumerate(LAYERS)
    ]
    betas = [
        nc.dram_tensor(f"b{l}", [E, dout], MM_DT, kind="ExternalInput")
        for l, (_, dout, _) in enumerate(LAYERS)
    ]
    outt = nc.dram_tensor("outt", [DIMS[3], B_LOC], F32, kind="ExternalOutput")

    with tile.TileContext(nc) as tc:
        with (
            tc.tile_pool(name="htp", bufs=12) as htp,
            tc.tile_pool(name="ztp", bufs=12) as ztp,
            tc.tile_pool(name="wp", bufs=14) as wp,
            tc.tile_pool(name="wbbp", bufs=8) as wbbp,
            tc.tile_pool(name="consts", bufs=1) as consts,
            tc.tile_pool(name="betap", bufs=2) as betap,
            tc.tile_pool(name="tmp", bufs=3) as tmp,
            tc.tile_pool(name="psp", bufs=8, space="PSUM") as psp,
        ):
            # --- startup ---
            # DMA *issue* costs ~640ns per dma_start on a sequencer, so the
            # critical first tiles are split across partitions AND issued on
            # three different engines' queues in parallel; the bulk weight
            # stream stays on nc.sync.
            # Critical path to the first real matmul: xt[0], wbb[0], pre_w[0].
            pre_w = []
            for j in range(DIMS[0] // P):
                t = wp.tile([P, DIMS[1]], MM_DT, tag="w")
                nsp = 4 if j == 0 else 2
                for q in range(nsp):
                    step = P // nsp
                    nc.sync.dma_start(
                        out=t[q * step : (q + 1) * step, :],
                        in_=ats[0][0, j * P + q * step : j * P + (q + 1) * step, :],
                    )
                pre_w.append(t)
            # x^T tiles (scalar queue) and blend weights (gpsimd queue)
            ht = []
            for j in range(DIMS[0] // P):
                t = htp.tile([P, B_LOC], F32, tag="ht")
                nsp = 4 if j == 0 else 2
                for q in range(nsp):
                    step = P // nsp
                    nc.scalar.dma_start(
                        out=t[q * step : (q + 1) * step, :],
                        in_=xt[j * P + q * step : j * P + (q + 1) * step, :],
                    )
                ht.append(t)
            wbb = []
            for e in range(E):
                t = wbbp.tile([P, B_LOC], F32, tag="wbb")
                nsp = 4 if e == 0 else (2 if e < 4 else 1)
                for q in range(nsp):
                    step = P // nsp
                    nc.gpsimd.dma_start(
                        out=t[q * step : (q + 1) * step, :],
                        in_=wbbd[e, q * step : (q + 1) * step, :],
                    )
                wbb.append(t)
            # wb as [E, B_LOC] tile: rhs of the (end-of-layer) bias matmuls
            wb_all = consts.tile([E, B_LOC], MM_DT, tag="wb_all")
            nc.gpsimd.dma_start(out=wb_all, in_=wb[:, :])

            # PE warm-up: the HAM clock gate needs ~3.4us of sustained PE
            # activity to lift the PE from 1.2 to 2.4 GHz. Burn the DMA-wait
            # window on junk matmuls into a scratch PSUM bank so the real
            # matmul stream starts warm.
            junk = consts.tile([P, B_LOC], F16, tag="junk")
            nc.vector.memset(junk, 0.0)
            warm_ps = psp.tile([P, B_LOC], F32, tag="ps")
            for _ in range(18):
                nc.tensor.matmul(
                    warm_ps, junk[:, :P], junk, start=True, stop=True
                )

            # --- layers ---
            for l, (din, dout, use_act) in enumerate(LAYERS):
                ni, no = din // P, dout // P
                beta_sb = betap.tile([E, dout], MM_DT, tag="beta")
                nc.gpsimd.dma_start(out=beta_sb, in_=betas[l][:, :])

                psums = []
                for _ in range(no):
                    pt = psp.tile([P, B_LOC], F32, tag="ps", name="ps")
                    psums.append(pt)

                # accumulate all experts; first expert opens each bank
                for e in range(E):
                    for j in range(ni):
                        zt = ztp.tile([P, B_LOC], MM_DT, tag="zt")
                        if l == 0:
                            nc.vector.tensor_mul(zt, ht[j], wbb[e])
                        else:
                            # ht holds elu(x)+1; fold the -1 into the blend
                            nc.vector.scalar_tensor_tensor(
                                zt, ht[j], -1.0, wbb[e], ALU.add, ALU.mult
                            )
                        if l == 0 and e == 0:
                            at_sb = pre_w[j]
                        else:
                            at_sb = wp.tile([P, dout], MM_DT, tag="w")
                            nc.sync.dma_start(
                                out=at_sb, in_=ats[l][e, j * P : (j + 1) * P, :]
                            )
                        final = e == E - 1 and j == ni - 1
                        for c in range(no):
                            nc.tensor.matmul(
                                psums[c],
                                at_sb[:, c * P : (c + 1) * P],
                                zt,
                                start=(e == 0 and j == 0),
                                stop=False,
                            )
                            if final:
                                # blended bias immediately closes each bank
                                # (accumulation is commutative; doing it at
                                # the end keeps the layer-start critical path
                                # free of the beta/wb DMAs, and interleaving
                                # it here lets bank c's eviction start while
                                # bank c+1 is still accumulating)
                                nc.tensor.matmul(
                                    psums[c],
                                    beta_sb[:, c * P : (c + 1) * P],
                                    wb_all,
                                    start=False,
                                    stop=True,
                                )

                # evict: elu(x)+1 for layers 0/1, direct DMA out for layer 2
                if use_act:
                    new_ht = []
                    for c in range(no):
                        r = tmp.tile([P, B_LOC], F32, tag="relu")
                        x = tmp.tile([P, B_LOC], F32, tag="expz")
                        h = htp.tile([P, B_LOC], F32, tag="ht")
                        nc.scalar.activation(r, psums[c], AF.Relu, scale=DESCALE)
                        nc.scalar.activation(x, psums[c], AF.Exp, scale=DESCALE)
                        # h = min(x, 1) + r  ( = elu + 1 )
                        nc.vector.scalar_tensor_tensor(h, x, 1.0, r, ALU.min, ALU.add)
                        new_ht.append(h)
                    ht = new_ht
                else:
                    for c in range(no):
                        # descale-copy on DVE (idle at kernel end), leaving
                        # Scalar/GpSimd free to issue the output DMAs
                        o = tmp.tile([P, B_LOC], F32, tag="out")
                        nc.vector.tensor_scalar_mul(o, psums[c], DESCALE)
                        # split the store across queues and engines: one
                        # 256KB DMA has ~13us latency and would dominate the
                        # kernel tail
                        for q in range(4):
                            eng = nc.gpsimd if q % 2 else nc.scalar
                            eng.dma_start(
                                out=outt[c * P + q * 32 : c * P + (q + 1) * 32, :],
                                in_=o[q * 32 : (q + 1) * 32, :],
                            )

    nc.compile()
    return nc


def _maybe_reset_device():
    """Clear stale NRT state on the axon terminal left by a crashed prior
    process. Only safe/needed before this process initializes its jax
    backend, and must run in a subprocess (CDLL'ing the axon .so in-process
    conflicts with jax's own dlopen)."""
    try:
        import jax._src.xla_bridge as xb

        if getattr(xb, "_backends", None):
            return  # backend already live in this process; don't touch it
    except Exception:
        pass
    try:
        import subprocess

        subprocess.run(
            [
                sys.executable,
                "-c",
                "import ctypes; lib = ctypes.CDLL('/opt/axon/libaxon_pjrt.so'); "
                "lib.axon_reset.restype = ctypes.c_int64; lib.axon_reset()",
            ],
            timeout=60,
            capture_output=True,
        )
    except Exception:
        pass


def kernel(x, weight_blend, a0, b0, a1, b1, a2, b2):
    global LAST_RESULTS, _NC_CACHE
    _maybe_reset_device()
    mm_np = np.float16 if MM_MODE == "f16" else np.float32
    x = np.ascontiguousarray(np.asarray(x, dtype=np.float32))
    weight_blend = np.ascontiguousarray(np.asarray(weight_blend, dtype=np.float32))
    aT = [
        np.ascontiguousarray(
            (np.asarray(a, dtype=np.float32) * float(2.0**WEXP))
            .transpose(0, 2, 1)
            .astype(mm_np)
        )
        for a in (a0, a1, a2)
    ]
    bs = [
        np.ascontiguousarray(
            (np.asarray(b, dtype=np.float32) * float(2.0 ** (WEXP + ZEXP))).astype(
                mm_np
            )
        )
        for b in (b0, b1, b2)
    ]

    if _NC_CACHE is None:
        _NC_CACHE = _build()
    nc = _NC_CACHE

    in_maps = []
    for c in range(N_CORES):
        sl = slice(c * B_LOC, (c + 1) * B_LOC)
        wb_c = np.ascontiguousarray(weight_blend[:, sl]) * float(2.0**ZEXP)
        in_maps.append(
            {
                "xt": np.ascontiguousarray(x[sl].T),
                "wb": wb_c.astype(mm_np),
                "wbb": np.ascontiguousarray(
                    np.broadcast_to(wb_c[:, None, :], (E, P, B_LOC))
                ),
                "a0t": aT[0],
                "a1t": aT[1],
                "a2t": aT[2],
                "b0": bs[0],
                "b1": bs[1],
                "b2": bs[2],
            }
        )

    trace = os.environ.get("BASS_KERNEL_TRACE") == "1"
    res = run_bass_kernel_spmd(
        nc, in_maps, core_ids=list(range(N_CORES)), trace=trace
    )
    LAST_RESULTS = res
    return np.concatenate(
        [np.asarray(r["outt"]).T for r in res.results], axis=0
    ).astype(np.float32)
